# revision 1
# baseline (speedup 1.0000x reference)
"""Trainium2 Bass kernel for nn_Block_8177617731884 (attention + MoE block).

Strategy (8 cores):
  - Attention: head-parallel (2 of 16 heads per core). Scores computed
    transposed (sT[tk,tq]) so softmax denom + o^T = v^T sT need no on-chip
    transposes; denominator fused into the AV matmul via a ones-column in v.
    q/k/v projections are folded into the rmsnorm(x) streaming loop so PE
    work hides under the x DMA.
  - ReduceScatter(attention partials) chunked per 512-token group and
    overlapped with attention compute -> per-core feature shard of attn out;
    add x shard -> x2 shard (f32).
  - Router logits + rmsnorm2 sum-of-squares are computed in f32 from the
    local feature shards and combined with one tiny AllReduce ([9, N]) --
    routing sees f32 logits, which controls the top-2 expert flips that
    dominate the error budget.
  - AllGather of bf16 x2 shards -> full x2 everywhere; xn2 = x2 * rstd.
  - MoE: expert-parallel (expert c on core c), dense over all tokens in
    bf16, weighted by on-device top-2 combine weights; bf16 chunked
    ReduceScatter overlapped with expert compute.
  - All activations feature-major ([embd, tokens]) so matmul contractions
    sit on partitions. Host pre-transposes x and pre-casts weights to bf16.
Output assembled on host from the 8 feature shards.
"""

import sys

import numpy as np

for _p in ("/opt/trn_rl_repo",):
    if _p not in sys.path:
        sys.path.insert(0, _p)

import ml_dtypes

B, T, C = 2, 2048, 1024
NH, HD = 16, 64
E, TOPK, H = 8, 2, 2048
N = B * T  # 4096 tokens
P = 128
NCORES = 8
EPS = 1e-5
NEG = -1e9
TT = 512  # token tile (free dim) for most matmuls
NTT = N // TT  # 8 token tiles
BTT = T // TT  # 4 token tiles per batch
PO = C // P  # 8 embd chunks
HC = H // P  # 16 expert-hidden chunks
KC = T // P  # 16 key chunks per batch
SC = 1.0 / np.sqrt(HD)

STAGES = {"rms1": 0, "attn": 1, "x2": 2, "router": 3, "full": 4, "notl": 4}

_CACHE = {}


def _build_nc(stage="full", tl=False):
    import concourse.bass as bass
    import concourse.mybir as mybir
    import concourse.tile as tile
    from concourse import bacc
    from concourse.bass import ts

    lvl = STAGES[stage]
    tl = tl or stage == "notl"

    dt = mybir.dt
    f32 = dt.float32
    bf16 = dt.bfloat16
    AF = mybir.ActivationFunctionType
    OP = mybir.AluOpType
    AX = mybir.AxisListType

    nc = bacc.Bacc(
        "TRN2", target_bir_lowering=False, debug=False, num_devices=NCORES
    )

    # ---- I/O ----
    xTb_h = nc.dram_tensor("xTb", [C, N], bf16, kind="ExternalInput")
    xTs_h = nc.dram_tensor("xTs", [P, N], f32, kind="ExternalInput")
    wq_h = nc.dram_tensor("wq", [C, P], bf16, kind="ExternalInput")
    wk_h = nc.dram_tensor("wk", [C, P], bf16, kind="ExternalInput")
    wv_h = nc.dram_tensor("wv", [C, P], bf16, kind="ExternalInput")
    wo_h = nc.dram_tensor("wo", [P, C], bf16, kind="ExternalInput")
    rws_h = nc.dram_tensor("rws", [P, E], f32, kind="ExternalInput")
    fc1_h = nc.dram_tensor("fc1", [C, H], bf16, kind="ExternalInput")
    fc2_h = nc.dram_tensor("fc2", [H, C], bf16, kind="ExternalInput")
    esel_h = nc.dram_tensor("esel", [P, E], f32, kind="ExternalInput")
    id8_h = nc.dram_tensor("id8", [E, E], f32, kind="ExternalInput")
    cmask_h = nc.dram_tensor("cmask", [4, P, TT], f32, kind="ExternalInput")
    out_h = nc.dram_tensor("out", [P, N], f32, kind="ExternalOutput")

    rg = [list(range(NCORES))]

    with tile.TileContext(nc) as tc:

        def collective(kind, op, in_ap, out_ap):
            if tl:
                if kind == "AllGather":
                    w = in_ap.shape[0]
                    for r in range(NCORES):
                        nc.sync.dma_start(
                            out=out_ap[r * w:(r + 1) * w], in_=in_ap
                        )
                elif kind == "ReduceScatter":
                    nc.sync.dma_start(out=out_ap, in_=in_ap[0:out_ap.shape[0]])
                else:  # AllReduce
                    nc.sync.dma_start(out=out_ap, in_=in_ap)
            else:
                nc.gpsimd.collective_compute(
                    kind, op, replica_groups=rg,
                    ins=[in_ap.opt()], outs=[out_ap.opt()],
                )

        with (
            tc.tile_pool(name="consts", bufs=1) as consts,
            tc.tile_pool(name="big", bufs=1) as big,
            tc.tile_pool(name="small", bufs=4) as small,
            tc.tile_pool(name="dram", bufs=1, space="DRAM") as dram,
        ):
            # ---- small constant preloads ----
            wq_s = consts.tile([P, PO, P], bf16, tag="wq")
            wk_s = consts.tile([P, PO, P], bf16, tag="wk")
            wv_s = consts.tile([P, PO, P], bf16, tag="wv")
            for wdst, wsrc in ((wq_s, wq_h), (wk_s, wk_h), (wv_s, wv_h)):
                nc.sync.dma_start(
                    out=wdst, in_=wsrc.ap().rearrange("(po pi) m -> pi po m", pi=P)
                )
            wo_s = consts.tile([P, C], bf16, tag="wo")
            nc.sync.dma_start(out=wo_s, in_=wo_h[:, :])
            rws_s = consts.tile([P, E], f32, tag="rws")
            nc.sync.dma_start(out=rws_s, in_=rws_h[:, :])
            cmask_s = consts.tile([P, 4, TT], f32, tag="cmask")
            nc.sync.dma_start(
                out=cmask_s, in_=cmask_h.ap().rearrange("d p f -> p d f")
            )
            esel_s = consts.tile([P, E], f32, tag="esel")
            nc.sync.dma_start(out=esel_s, in_=esel_h[:, :])
            ones_b = consts.tile([P, 1], bf16, tag="ones")
            nc.vector.memset(ones_b, 1.0)
            eps_s = consts.tile([1, 1], f32, tag="eps")
            nc.vector.memset(eps_s, EPS)
            id8 = consts.tile([E, E], f32, tag="id8")
            nc.sync.dma_start(out=id8, in_=id8_h[:, :])

            # DRAM intermediates
            attn_part = dram.tile([NTT, C, TT], f32, tag="attn_part")
            attn_shard = dram.tile([NTT, P, TT], f32, tag="attn_shard")
            x2s_d = dram.tile([P, N], f32, tag="x2s")
            x2bf_s = dram.tile([P, N], bf16, tag="x2bf_s")
            x2bf_full = dram.tile([C, N], bf16, tag="x2bf_full")
            lgssq_part = dram.tile([E + 1, N], f32, tag="lgssq_part")
            lgssq_ar = dram.tile([E + 1, N], f32, tag="lgssq_ar")
            moe_part = dram.tile([NTT, C, TT], bf16, tag="moe_part")
            moe_shard = dram.tile([NTT, P, TT], bf16, tag="moe_shard")
            cc_dram = dram.tile([1, N], f32, tag="cc")

            xTb_v = xTb_h.ap().rearrange("(po pi) n -> pi po n", pi=P)

            # ===== Stage 1: rmsnorm(x) fused with q/k/v projections =====
            xnT = big.tile([P, PO, N], bf16, tag="xn", name="xnT")
            with tc.tile_pool(name="qkvp", bufs=1) as qkvp:
              qT = qkvp.tile([P, NTT, TT], bf16, tag="qT")
              kT = qkvp.tile([P, NTT, TT], bf16, tag="kT")
              v_aug = qkvp.tile([P, B, KC, 2, HD + 2], bf16, tag="v_aug")
              nc.vector.memset(v_aug[:, :, :, :, HD:HD + 1], 1.0)
              with tc.tile_pool(name="rms1", bufs=3) as rms1, \
                   tc.tile_pool(name="ps1", bufs=2, space="PSUM") as ps1:
                for t in range(NTT):
                    b, qt = t // BTT, t % BTT
                    xt = rms1.tile([P, PO, TT], bf16, tag="xt")
                    nc.sync.dma_start(out=xt, in_=xTb_v[:, :, ts(t, TT)])
                    ssq = ps1.tile([1, TT], f32, tag="ssq", name="ssq")
                    for po in range(PO):
                        sq = rms1.tile([P, TT], bf16, tag="sq", name=f"sq{po}")
                        nc.vector.tensor_mul(sq, xt[:, po], xt[:, po])
                        nc.tensor.matmul(
                            ssq, lhsT=ones_b, rhs=sq,
                            start=(po == 0), stop=(po == PO - 1),
                        )
                    rstd = rms1.tile([1, TT], f32, tag="rstd")
                    nc.scalar.activation(
                        out=rstd, in_=ssq, func=AF.Sqrt, bias=eps_s, scale=1.0 / C
                    )
                    nc.vector.reciprocal(rstd, rstd)
                    rstd_b = rms1.tile([P, TT], f32, tag="rstd_b")
                    nc.gpsimd.partition_broadcast(rstd_b, rstd)
                    for po in range(PO):
                        nc.vector.tensor_tensor(
                            xnT[:, po, ts(t, TT)], xt[:, po], rstd_b, OP.mult
                        )
                    # q/k for this token tile
                    for wsb, dst, scl, pstag in (
                        (wq_s, qT, SC, "q"), (wk_s, kT, 1.0, "k")
                    ):
                        ps = ps1.tile([P, TT], f32, tag=pstag)
                        for po in range(PO):
                            nc.tensor.matmul(
                                ps, lhsT=wsb[:, po], rhs=xnT[:, po, ts(t, TT)],
                                start=(po == 0), stop=(po == PO - 1),
                            )
                        nc.scalar.activation(
                            out=dst[:, t], in_=ps, func=AF.Copy, scale=scl
                        )
                    # v (token-major) for the 4 key chunks of this tile
                    for j in range(4):
                        kc = qt * 4 + j
                        tok0 = t * TT + j * P
                        ps = ps1.tile([P, P], f32, tag="v")
                        for po in range(PO):
                            nc.tensor.matmul(
                                ps, lhsT=xnT[:, po, tok0:tok0 + P],
                                rhs=wv_s[:, po],
                                start=(po == 0), stop=(po == PO - 1),
                            )
                        for h in range(2):
                            nc.scalar.copy(
                                v_aug[:, b, kc, h, 0:HD],
                                ps[:, h * HD:(h + 1) * HD],
                            )

              if lvl == 0:
                  with tc.tile_pool(name="dbg", bufs=2) as dbg:
                      for t in range(NTT):
                          d_t = dbg.tile([P, TT], f32, tag="d")
                          nc.vector.tensor_copy(d_t, xnT[:, 0, ts(t, TT)])
                          nc.sync.dma_start(out=out_h[:, ts(t, TT)], in_=d_t)

              # ================= Stage 2: attention =================
              if lvl >= 1:
                with tc.tile_pool(name="attn", bufs=1) as attn, \
                     tc.tile_pool(name="attw", bufs=3) as attw, \
                     tc.tile_pool(name="expw", bufs=17) as expw, \
                     tc.tile_pool(name="psAtt", bufs=2, space="PSUM") as psAtt, \
                     tc.tile_pool(name="res", bufs=2) as res:
                  for b in range(B):
                      oT = attn.tile([P, BTT, TT], bf16, tag="oT")
                      for qt in range(BTT):
                          cq = b * BTT + qt  # global 512-token chunk id
                          nkc = 4 * qt + 4
                          exps = []
                          for kc in range(nkc):
                              spair = psAtt.tile([P, 2, TT], f32, tag="spair",
                                                 name=f"s_{kc}")
                              for h in range(2):
                                  nc.tensor.matmul(
                                      spair[:, h],
                                      lhsT=kT[h * HD:(h + 1) * HD,
                                              b * BTT + kc // 4,
                                              (kc % 4) * P:(kc % 4 + 1) * P],
                                      rhs=qT[h * HD:(h + 1) * HD, cq],
                                      start=True, stop=True,
                                  )
                              d = kc - 4 * qt
                              if d >= 0:
                                  nc.vector.tensor_tensor(
                                      spair, spair,
                                      cmask_s[:, d:d + 1, :].to_broadcast(
                                          (P, 2, TT)),
                                      OP.add,
                                  )
                              ep = expw.tile([P, 2, TT], bf16, tag="exp",
                                             name=f"e_{kc}")
                              nc.scalar.activation(ep, spair, AF.Exp)
                              exps.append(ep)
                          ops_h = [
                              psAtt.tile([P, TT], f32, tag="o", name=f"o_{h}")
                              for h in range(2)
                          ]
                          for kc in range(nkc):
                              for h in range(2):
                                  nc.tensor.matmul(
                                      ops_h[h][0:HD + 1],
                                      lhsT=v_aug[:, b, kc, h, 0:HD + 1],
                                      rhs=exps[kc][:, h],
                                      start=(kc == 0), stop=(kc == nkc - 1),
                                  )
                          for h in range(2):
                              rec = attw.tile([1, TT], f32, tag="rec")
                              nc.vector.reciprocal(rec, ops_h[h][HD:HD + 1])
                              rec_b = attw.tile([HD, TT], f32, tag="rec_b")
                              nc.gpsimd.partition_broadcast(rec_b, rec)
                              nc.vector.tensor_tensor(
                                  oT[h * HD:(h + 1) * HD, qt], ops_h[h][0:HD],
                                  rec_b, OP.mult,
                              )
                          # wo projection for this chunk, then RS it
                          for dc in range(PO):
                              aps = psAtt.tile([P, TT], f32, tag="mmA")
                              nc.tensor.matmul(
                                  aps, lhsT=wo_s[:, dc * P:(dc + 1) * P],
                                  rhs=oT[:, qt], start=True, stop=True,
                              )
                              asb = attw.tile([P, TT], f32, tag="asb")
                              if dc % 2 == 0:
                                  nc.scalar.copy(asb, aps)
                              else:
                                  nc.vector.tensor_copy(asb, aps)
                              nc.sync.dma_start(
                                  out=attn_part[cq, dc * P:(dc + 1) * P, :],
                                  in_=asb,
                              )
                          if lvl >= 2:
                              collective(
                                  "ReduceScatter", mybir.AluOpType.add,
                                  attn_part[cq], attn_shard[cq],
                              )
                              # residual + shard-local router partials
                              a_t = res.tile([P, TT], f32, tag="res_a")
                              x_t = res.tile([P, TT], f32, tag="res_x")
                              nc.sync.dma_start(out=a_t, in_=attn_shard[cq])
                              nc.sync.dma_start(
                                  out=x_t, in_=xTs_h[:, ts(cq, TT)]
                              )
                              nc.vector.tensor_add(a_t, a_t, x_t)
                              nc.sync.dma_start(
                                  out=x2s_d[:, ts(cq, TT)], in_=a_t
                              )
                              ab = res.tile([P, TT], bf16, tag="res_ab")
                              nc.vector.tensor_copy(ab, a_t)
                              nc.sync.dma_start(
                                  out=x2bf_s[:, ts(cq, TT)], in_=ab
                              )
                              lp = psAtt.tile([40, TT], f32, tag="mmA",
                                              name="lp")
                              nc.tensor.matmul(
                                  lp[0:E], lhsT=rws_s, rhs=a_t,
                                  start=True, stop=True,
                              )
                              sqs = res.tile([P, TT], bf16, tag="res_sq")
                              nc.vector.tensor_mul(sqs, a_t, a_t)
                              nc.tensor.matmul(
                                  lp[32:33], lhsT=ones_b, rhs=sqs,
                                  start=True, stop=True,
                              )
                              lsb = res.tile([E, TT], f32, tag="res_l")
                              nc.scalar.copy(lsb, lp[0:E])
                              sq_l = res.tile([1, TT], f32, tag="res_ssq")
                              nc.scalar.copy(sq_l, lp[32:33])
                              nc.sync.dma_start(
                                  out=lgssq_part[0:E, ts(cq, TT)], in_=lsb
                              )
                              nc.sync.dma_start(
                                  out=lgssq_part[E:E + 1, ts(cq, TT)],
                                  in_=sq_l,
                              )

            if stage == "attn":
                with tc.tile_pool(name="dbg", bufs=2) as dbg:
                    for t in range(NTT):
                        d_t = dbg.tile([P, TT], f32, tag="d")
                        nc.sync.dma_start(out=d_t, in_=attn_part[t, 0:P, :])
                        nc.sync.dma_start(out=out_h[:, ts(t, TT)], in_=d_t)

            if lvl >= 2:
                collective("AllReduce", mybir.AluOpType.add,
                           lgssq_part, lgssq_ar)
                collective("AllGather", mybir.AluOpType.bypass,
                           x2bf_s, x2bf_full)

            if lvl == 2:
                with tc.tile_pool(name="dbg", bufs=2) as dbg:
                    for t in range(NTT):
                        d_t = dbg.tile([P, TT], f32, tag="d")
                        nc.sync.dma_start(out=d_t, in_=x2s_d[:, ts(t, TT)])
                        nc.sync.dma_start(out=out_h[:, ts(t, TT)], in_=d_t)

            # ============ Stage 3: rmsnorm(x2) + router ============
            nj = N // P
            if lvl >= 3:
              xn2T = big.tile([P, PO, N], bf16, tag="xn", name="xn2T")
              moec_cm = tc.tile_pool(name="moec", bufs=1) if lvl >= 4 else None
              moec = moec_cm.__enter__() if moec_cm is not None else None
              if moec is not None:
                  fc1_s = moec.tile([P, PO, H], bf16, tag="fc1")
                  nc.sync.dma_start(
                      out=fc1_s,
                      in_=fc1_h.ap().rearrange("(po pi) h -> pi po h", pi=P),
                  )
                  fc2_s = moec.tile([P, HC, C], bf16, tag="fc2")
                  nc.sync.dma_start(
                      out=fc2_s,
                      in_=fc2_h.ap().rearrange("(hc hi) d -> hi hc d", hi=P),
                  )
              with tc.tile_pool(name="rms2", bufs=2) as rms2, \
                   tc.tile_pool(name="rout", bufs=1) as rout, \
                   tc.tile_pool(name="psB", bufs=2, space="PSUM") as psB:
                # rstd for all tokens from the AllReduced sum-of-squares
                rstd_all = rout.tile([1, N], f32, tag="rstd_all")
                nc.sync.dma_start(out=rstd_all, in_=lgssq_ar[E:E + 1, :])
                nc.scalar.activation(
                    out=rstd_all, in_=rstd_all, func=AF.Sqrt,
                    bias=eps_s, scale=1.0 / C,
                )
                nc.vector.reciprocal(rstd_all, rstd_all)
                # xn2 = x2(bf16) * rstd
                x2bf_v = x2bf_full[:].rearrange("(po pi) n -> pi po n", pi=P)
                for t in range(NTT):
                    xt = rms2.tile([P, PO, TT], bf16, tag="xt2")
                    nc.sync.dma_start(out=xt, in_=x2bf_v[:, :, ts(t, TT)])
                    rstd_b = rms2.tile([P, TT], f32, tag="rstd_b2")
                    nc.gpsimd.partition_broadcast(
                        rstd_b, rstd_all[0:1, ts(t, TT)]
                    )
                    for po in range(PO):
                        nc.vector.tensor_tensor(
                            xn2T[:, po, ts(t, TT)], xt[:, po], rstd_b, OP.mult
                        )
                # router logits: lg = lgraw * rstd  (feature-major [E, N])
                lg_sb = rout.tile([E, N], f32, tag="lg")
                nc.sync.dma_start(out=lg_sb, in_=lgssq_ar[0:E, :])
                for t in range(NTT):
                    rb8 = rms2.tile([E, TT], f32, tag="rb8")
                    nc.gpsimd.partition_broadcast(
                        rb8, rstd_all[0:1, ts(t, TT)]
                    )
                    nc.vector.tensor_tensor(
                        lg_sb[:, ts(t, TT)], lg_sb[:, ts(t, TT)], rb8, OP.mult
                    )
                # transpose to token-major [P, N/P, E] via PE transposes
                lt = rout.tile([P, nj, E], f32, tag="lt")
                for j in range(nj):
                    tps = psB.tile([P, E], f32, tag="tp", name=f"tp_{j}")
                    nc.tensor.transpose(tps, lg_sb[:, j * P:(j + 1) * P], id8)
                    nc.scalar.copy(lt[:, j], tps)
                # top-2 + combine weights (token-major)
                m1 = small.tile([P, nj, 1], f32, tag="m1")
                nc.vector.tensor_reduce(m1, lt, axis=AX.X, op=OP.max)
                eq = rout.tile([P, nj, E], f32, tag="eq")
                nc.vector.tensor_tensor(
                    eq, lt, m1.to_broadcast((P, nj, E)), OP.is_equal
                )
                lt2 = rout.tile([P, nj, E], f32, tag="lt2")
                nc.vector.scalar_tensor_tensor(
                    lt2, in0=eq, scalar=NEG, in1=lt, op0=OP.mult, op1=OP.add
                )
                m2 = small.tile([P, nj, 1], f32, tag="m2")
                nc.vector.tensor_reduce(m2, lt2, axis=AX.X, op=OP.max)
                sel = eq  # reuse scratch
                nc.vector.tensor_tensor(
                    sel, lt, m2.to_broadcast((P, nj, E)), OP.is_ge
                )
                dmx = lt2  # reuse scratch
                nc.vector.tensor_tensor(
                    dmx, lt, m1.to_broadcast((P, nj, E)), OP.subtract
                )
                ex = dmx
                nc.scalar.activation(ex, dmx, AF.Exp)
                ws = ex
                nc.vector.tensor_tensor(ws, ex, sel, OP.mult)
                den = small.tile([P, nj, 1], f32, tag="den")
                nc.vector.tensor_reduce(den, ws, axis=AX.X, op=OP.add)
                rden = small.tile([P, nj, 1], f32, tag="rden")
                nc.vector.reciprocal(rden, den)
                wse = ws
                nc.vector.tensor_tensor(
                    wse, ws, esel_s.unsqueeze(1).to_broadcast((P, nj, E)), OP.mult
                )
                cc_tok = small.tile([P, nj], f32, tag="cc_tok")
                nc.vector.tensor_reduce(cc_tok, wse, axis=AX.X, op=OP.add)
                nc.vector.tensor_tensor(cc_tok, cc_tok, rden[:, :, 0], OP.mult)
                # flatten to [1, N] (free-major) via DRAM round-trip
                nc.sync.dma_start(
                    out=cc_dram[0].rearrange("(j p) -> p j", p=P), in_=cc_tok
                )

              if lvl == 3:
                with tc.tile_pool(name="dbg", bufs=2) as dbg:
                    cfr = dbg.tile([1, N], f32, tag="cfr")
                    nc.sync.dma_start(out=cfr, in_=cc_dram[0:1, :])
                    cfb = dbg.tile([P, N], f32, tag="cfb")
                    nc.gpsimd.partition_broadcast(cfb, cfr)
                    nc.sync.dma_start(out=out_h[:, :], in_=cfb)

            # ================= Stage 4: dense expert =================
            if lvl >= 4:
              with tc.tile_pool(name="moe", bufs=2) as moe, \
                   tc.tile_pool(name="moew", bufs=3) as moew, \
                   tc.tile_pool(name="psM", bufs=2, space="PSUM") as psM:
                for t in range(NTT):
                    cf_t = moew.tile([1, TT], f32, tag="cf")
                    nc.sync.dma_start(out=cf_t, in_=cc_dram[0:1, ts(t, TT)])
                    cf_b = moew.tile([P, TT], f32, tag="cf_b")
                    nc.gpsimd.partition_broadcast(cf_b, cf_t)
                    h1 = moe.tile([P, HC, TT], bf16, tag="h1")
                    for hc in range(HC):
                        hps = psM.tile([P, TT], f32, tag="hps")
                        for po in range(PO):
                            nc.tensor.matmul(
                                hps, lhsT=fc1_s[:, po, hc * P:(hc + 1) * P],
                                rhs=xn2T[:, po, ts(t, TT)],
                                start=(po == 0), stop=(po == PO - 1),
                            )
                        r = moew.tile([P, TT], bf16, tag="relu")
                        nc.scalar.activation(r, hps, AF.Relu)
                        nc.vector.tensor_mul(h1[:, hc], r, r)
                    for dc in range(PO):
                        ops_ = psM.tile([P, TT], f32, tag="ops")
                        for hc in range(HC):
                            nc.tensor.matmul(
                                ops_, lhsT=fc2_s[:, hc, dc * P:(dc + 1) * P],
                                rhs=h1[:, hc],
                                start=(hc == 0), stop=(hc == HC - 1),
                            )
                        osb = moew.tile([P, TT], bf16, tag="osb")
                        nc.vector.tensor_tensor(osb, ops_, cf_b, OP.mult)
                        nc.sync.dma_start(
                            out=moe_part[t, dc * P:(dc + 1) * P, :], in_=osb
                        )
                    collective("ReduceScatter", mybir.AluOpType.add,
                               moe_part[t], moe_shard[t])

              # ============ final residual ============
              with tc.tile_pool(name="fin", bufs=2) as fin:
                for t in range(NTT):
                    m_t = fin.tile([P, TT], bf16, tag="fin_m")
                    x_t = fin.tile([P, TT], f32, tag="fin_x")
                    o_t = fin.tile([P, TT], f32, tag="fin_o")
                    nc.sync.dma_start(out=m_t, in_=moe_shard[t])
                    nc.sync.dma_start(out=x_t, in_=x2s_d[:, ts(t, TT)])
                    nc.vector.tensor_tensor(o_t, m_t, x_t, OP.add)
                    nc.sync.dma_start(out=out_h[:, ts(t, TT)], in_=o_t)

              if moec_cm is not None:
                  moec_cm.__exit__(None, None, None)

    nc.finalize()
    return nc


def _prep_inputs(x, wq, wk, wv, wo, router_w, exp_fc1, exp_fc2):
    bf = ml_dtypes.bfloat16
    xT = np.ascontiguousarray(x.reshape(N, C).T.astype(np.float32))
    wq_b = wq.astype(bf)
    wk_b = wk.astype(bf)
    wv_b = wv.astype(bf)
    wo_b = wo.astype(bf)
    fc1_b = exp_fc1.astype(bf)
    fc2_b = exp_fc2.astype(bf)
    # causal mask-add tiles: cmask[d,p,f] = 0 if f >= p + 128*d else NEG
    dgrid = np.arange(4)[:, None, None] * P
    pgrid = np.arange(P)[None, :, None]
    fgrid = np.arange(TT)[None, None, :]
    cmask = np.where(fgrid >= pgrid + dgrid, 0.0, NEG).astype(np.float32)
    xTb = xT.astype(bf)

    in_maps = []
    for c in range(NCORES):
        esel = np.zeros((P, E), np.float32)
        esel[:, c] = 1.0
        in_maps.append({
            "xTb": xTb,
            "xTs": np.ascontiguousarray(xT[c * P:(c + 1) * P]),
            "wq": np.ascontiguousarray(wq_b[:, c * P:(c + 1) * P]),
            "wk": np.ascontiguousarray(wk_b[:, c * P:(c + 1) * P]),
            "wv": np.ascontiguousarray(wv_b[:, c * P:(c + 1) * P]),
            "wo": np.ascontiguousarray(wo_b[c * P:(c + 1) * P, :]),
            "rws": np.ascontiguousarray(
                router_w[c * P:(c + 1) * P, :].astype(np.float32)
            ),
            "fc1": np.ascontiguousarray(fc1_b[c]),
            "fc2": np.ascontiguousarray(fc2_b[c]),
            "esel": esel,
            "id8": np.eye(E, dtype=np.float32),
            "cmask": cmask,
        })
    return in_maps


def _get_nc():
    if "nc" not in _CACHE:
        _CACHE["nc"] = _build_nc()
    return _CACHE["nc"]


def kernel(x, wq, wk, wv, wo, router_w, exp_fc1, exp_fc2, _run_kwargs=None):
    from concourse.bass_utils import run_bass_kernel_spmd

    nc = _get_nc()
    in_maps = _prep_inputs(x, wq, wk, wv, wo, router_w, exp_fc1, exp_fc2)
    kw = dict(_run_kwargs or {})
    res = run_bass_kernel_spmd(nc, in_maps, core_ids=list(range(NCORES)), **kw)
    _CACHE["last_results"] = res
    shards = [res.results[c]["out"] for c in range(NCORES)]
    full = np.concatenate(shards, axis=0)  # [C, N] feature-major
    return np.ascontiguousarray(full.T).reshape(B, T, C).astype(np.float32)

